# revision 82
# baseline (speedup 1.0000x reference)
"""Multi-head attention (B=4,N=2048,C=768,H=12) on 8 trn2 NeuronCores.

Sharding: data-parallel over B (4 batches x 2 cores each), tensor-parallel
over heads (6 heads per core). Each core:
  - QKV projection for its 6 heads (bf16 inputs/weights, fp32 accumulate;
    bf16 x adds ~2e-3 rel err and halves the DMA fill on the critical path)
  - transposed scores st[kv, q] (f32r, contraction D=64), two heads
    row-packed into PE partitions 0-63 / 64-127
  - exp on ScalarE (scale fused), bf16 output
  - attn@V in [q, d] layout: est is the stationary operand and V (with a
    ones-column for the softmax denominator) moves, ap=65 — half the PE
    cost of the [d, q] form; the denominator lands on the free axis so
    normalize is a per-partition reciprocal + tensor_scalar_mul (no
    partition broadcast), and a small PE transpose (identity input) puts
    each head at its proj partitions (odd head at base 64, no shift DMA).
    Each accumulator runs as its OWN kt-loop in its own ring buffer:
    start=True zeroes beyond the written region, so interleaved
    accumulation groups in one PSUM bank corrupt each other (measured).
  - output projection (bf16) -> per-pair partial y (bf16) to DRAM
Host sums the six partials per batch (3 pairs x 2 cores) and adds bias.

Schedule (the TileScheduler pulls the highest-priority READY instruction
whenever an engine idles, so emission order is a priority schedule):
  - The exp stream on ScalarE is the critical path; every strip's
    scores+exp are emitted first (phase 1), and its attn@V+normalize
    (phase 2) is emitted one window later (software pipelining over a
    26-deep est ring), so a window's drain never outranks the next
    window's scores.
  - The DMA fill is strip-ordered with few large transfers (the issue
    front-end costs ~0.65us each); pair 0's attention chases the fill.
  - Chase work (K strips 4 kv-tiles ahead of their scores, q strips one
    window ahead) and proj units ride in indexed slots of the two
    streams; proj of pair N-1 fills pair N's exp-wait gaps.
  - The final strip is split 2x256 and the last y writes are 2-mt solo
    DMAs whose staging copies go to DVE and ACT in parallel.
"""

import sys

import numpy as np
import ml_dtypes

_REPO = "/opt/trn_rl_repo"
if _REPO not in sys.path:
    sys.path.insert(0, _REPO)

import concourse.bacc as bacc
import concourse.mybir as mybir
import concourse.tile as tile
from concourse.bass_utils import run_bass_kernel_spmd

B, N, C, H, D = 4, 2048, 768, 12, 64
HL = H // 2          # heads per core
SCALE = D ** -0.5
NCORES = 8
KT_C = C // 128      # 6 contraction tiles over C
QS = N // 512        # 4 query strips
KVT = N // 128       # 16 kv tiles

F32 = mybir.dt.float32
F32R = mybir.dt.float32r
BF16 = mybir.dt.bfloat16
EXP = mybir.ActivationFunctionType.Exp

_CACHE = {}


def _build():
    nc = bacc.Bacc("TRN2", target_bir_lowering=False, debug=False,
                   num_devices=NCORES)
    xT = nc.dram_tensor("xT", [C, N], BF16, kind="ExternalInput").ap()
    wqkT = nc.dram_tensor("wqkT", [C, 2 * HL * D], BF16, kind="ExternalInput").ap()
    wvT = nc.dram_tensor("wvT", [C, HL * D], BF16, kind="ExternalInput").ap()
    wpT = nc.dram_tensor("wpT", [HL * D, C], BF16, kind="ExternalInput").ap()
    identT = nc.dram_tensor("identT", [128, 128], BF16, kind="ExternalInput").ap()
    y = nc.dram_tensor("y", [HL // 2, N, C], BF16, kind="ExternalOutput").ap()

    with tile.TileContext(nc) as tc:
        with (
            tc.tile_pool(name="singles", bufs=1) as singles,
            tc.tile_pool(name="ps_a", bufs=2, space="PSUM") as ps_a,
            tc.tile_pool(name="ps_st", bufs=2, space="PSUM") as ps_st,
            tc.tile_pool(name="ps_out", bufs=2, space="PSUM") as ps_out,
            tc.tile_pool(name="est", bufs=26) as est_p,
            tc.tile_pool(name="rec", bufs=4) as rec_p,
            tc.tile_pool(name="rb", bufs=3) as rb_p,
            tc.tile_pool(name="ysb", bufs=2) as ysb_p,
        ):
            xT_sb = singles.tile([128, KT_C, N], BF16)
            wqk_sb = singles.tile([128, KT_C, 2 * HL * D], BF16)
            wv_sb = singles.tile([128, KT_C, HL * D], BF16)
            wp_sb = singles.tile([128, HL // 2, C], BF16)
            qk_sb = singles.tile([128, 2 * (HL // 2), N], F32R)
            # per head: [V | ones]; the ones column yields the softmax denom
            v_sb = singles.tile([128, KVT, HL // 2, 2, D + 1], BF16)
            # attention output in proj-ready pair layout: [128, pair, N]
            attn_sb = singles.tile([128, HL // 2, N], BF16)
            # pair-2 odd head's proj rows replicated at partitions 0-63 so
            # the final sub-strip can project without the partition-shift
            # DMA (split-K accumulation instead)
            wp_odd_sb = singles.tile([64, C], BF16)
            ident_sb = singles.tile([128, 128], BF16)
            # scratch for the PE warmup matmuls; memset before the DMAs so
            # the warmups only wait on this one short DVE op
            nc.vector.memset(attn_sb[:, 0, 0:640], 0.0)

            # --- DMA fill, strip-ordered so attention pair 0 can chase it.
            # Few, large transfers: the DMA descriptor front-end costs
            # ~0.6us per dma_start regardless of size. wqk slices for the
            # k-tile (t=3) and q-tile (t=0) of pair 0 come first.
            t0c, t3c = 0, (HL // 2) * 128
            wqkT_k = wqkT.rearrange("(kt p) c -> p kt c", p=128)
            wvT_k = wvT.rearrange("(kt p) c -> p kt c", p=128)
            nc.sync.dma_start(wqk_sb[:, :, t3c:t3c + 128],
                              wqkT_k[:, :, t3c:t3c + 128])
            # each xT strip is one transfer; the DMA issue front-end costs
            # ~0.65us per dma_start, so fewer issues beat finer chase
            # granularity for time-to-first-exp
            xT_k = xT.rearrange("(kt p) n -> p kt n", p=128)
            nc.sync.dma_start(xT_sb[:, :, 0:512], xT_k[:, :, 0:512])
            nc.sync.dma_start(wqk_sb[:, :, t0c:t0c + 128],
                              wqkT_k[:, :, t0c:t0c + 128])
            nc.sync.dma_start(wv_sb, wvT_k)
            for s in range(1, QS):
                sl = slice(s * 512, (s + 1) * 512)
                nc.sync.dma_start(xT_sb[:, :, sl], xT_k[:, :, sl])
            nc.sync.dma_start(wqk_sb[:, :, 128:384], wqkT_k[:, :, 128:384])
            nc.sync.dma_start(wqk_sb[:, :, 512:768], wqkT_k[:, :, 512:768])
            nc.sync.dma_start(wp_sb, wpT.rearrange("(pr p) c -> p pr c", p=128))
            nc.sync.dma_start(wp_odd_sb, wpT[2 * 128 + 64:3 * 128, :])
            nc.sync.dma_start(ident_sb, identT)
            nc.vector.memset(v_sb[:, :, :, :, D:D + 1], 1.0)

            # warm the ACT exp table so the ~1.3us ACT_TABLE_LOAD is off the
            # first real exp's critical path
            warm_in = rec_p.tile([1, 2], F32, tag="warm")
            warm_out = rec_p.tile([1, 2], BF16, tag="warmo")
            nc.vector.memset(warm_in, 0.0)
            nc.scalar.activation(warm_out, warm_in, EXP, scale=SCALE)

            # dependency-free matmuls ramp the PE clock to 2.4GHz during the
            # initial DMA window so the first real matmuls run at full speed
            for _ in range(7):
                warm_ps = ps_out.tile([128, 512], F32, tag="out")
                nc.tensor.matmul(warm_ps, lhsT=attn_sb[:, 0, 0:128],
                                 rhs=attn_sb[:, 0, 128:640])

            def emit_qk_strip(t, qs):
                qsl = slice(qs * 512, (qs + 1) * 512)
                ps = ps_a.tile([128, 512], F32, tag="ps_a")
                for kt in range(KT_C):
                    nc.tensor.matmul(
                        ps,
                        lhsT=wqk_sb[:, kt, t * 128:(t + 1) * 128],
                        rhs=xT_sb[:, kt, qsl],
                        start=(kt == 0), stop=(kt == KT_C - 1),
                    )
                nc.vector.tensor_copy(qk_sb[:, t, qsl], ps)

            def emit_v(mt):
                ps = ps_a.tile([128, HL * D], F32, tag="ps_a")
                for kt in range(KT_C):
                    nc.tensor.matmul(
                        ps,
                        lhsT=xT_sb[:, kt, mt * 128:(mt + 1) * 128],
                        rhs=wv_sb[:, kt, :],
                        start=(kt == 0), stop=(kt == KT_C - 1),
                    )
                nc.vector.tensor_copy(
                    v_sb[:, mt, :, :, 0:D],
                    ps.rearrange("p (pr two d) -> p pr two d", pr=HL // 2, two=2),
                )

            stg_out = {}

            def emit_scores_exp(pr, qs, c0=0, cw=512, slots1=None):
                # scores + exp for every kv tile of query sub-range
                # [c0, c0+cw) of strip qs. The exp stream is the kernel's
                # critical path; slots1 interleaves the K/q-strip chase a
                # few kv tiles ahead of the scores that consume it.
                tq, tk = pr, HL // 2 + pr
                qsl = slice(qs * 512 + c0, qs * 512 + c0 + cw)
                ests = []
                for kt in range(KVT):
                    # both heads' scores into one 2-bank tile, one exp
                    st = ps_st.tile([128, 2, 512], F32, tag="st")
                    for half in range(2):
                        p0, p1 = half * 64, (half + 1) * 64
                        nc.tensor.matmul(
                            st[:, half, 0:cw],
                            lhsT=qk_sb[p0:p1, tk, kt * 128:(kt + 1) * 128],
                            rhs=qk_sb[p0:p1, tq, qsl],
                        )
                    est = est_p.tile([128, 2, 512], BF16, tag="est")
                    nc.scalar.activation(est[:, :, 0:cw], st[:, :, 0:cw],
                                         EXP, scale=SCALE)
                    ests.append(est)
                    if slots1 and kt in slots1:
                        emit_slack(slots1[kt])
                return ests

            def emit_attnv_norm(pr, qs, ests, c0=0, cw=512,
                                skip_shift=False, slots=None):
                # attn@V in [q, d] layout: est is the stationary operand,
                # V moves (ap=65), so attn@V costs half of the [d, q]
                # form in PE time; the softmax denominator lands on the
                # FREE axis so normalize is a native per-partition
                # tensor_scalar_mul (no partition broadcast), and a small
                # PE transpose drops each head at its proj partitions
                # (odd head legally at base 64) with no shift DMA.
                # Each accumulator runs as its OWN kt-loop in its own
                # ring buffer: a matmul with start=True zeroes beyond the
                # written region, so interleaved accumulation groups in
                # one PSUM bank corrupt each other (measured on hardware).
                for j in range(cw // 128):
                    qcol = qs * 512 + c0 + j * 128
                    for half in range(2):
                        out_q = ps_out.tile([128, 512], F32, tag="out")
                        for kt in range(KVT):
                            nc.tensor.matmul(
                                out_q[:, 0:65],
                                lhsT=ests[kt][:, half,
                                              j * 128:(j + 1) * 128],
                                rhs=v_sb[:, kt, pr, half, :],
                                start=(kt == 0), stop=(kt == KVT - 1),
                            )
                        rec_q = rec_p.tile([128, 1], F32, tag="recq")
                        nc.vector.reciprocal(rec_q, out_q[:, 64:65])
                        aq = rb_p.tile([128, 64], BF16, tag="aq")
                        nc.vector.tensor_scalar_mul(aq, out_q[:, 0:64],
                                                    rec_q)
                        tp = ps_a.tile([128, 128], BF16, tag="ps_a",
                                       name="tp")
                        nc.tensor.transpose(
                            tp[half * 64:(half + 1) * 64, :], aq, ident_sb)
                        nc.vector.tensor_copy(
                            attn_sb[half * 64:(half + 1) * 64, pr,
                                    qcol:qcol + 128],
                            tp[half * 64:(half + 1) * 64, :])
                    if slots:
                        for kk in range(4 * j, 4 * j + 4):
                            if kk in slots:
                                emit_slack(slots[kk])

            # proj: one DMA per 4-mt strip (the y-write descriptor front-end
            # costs ~0.6us per dma_start, so per-mt writes would throttle
            # the tail); a proj "unit" is 2 mt tiles, two units share a ysb
            ysb_open = {}

            def emit_proj_unit(pr, u, solo=False):
                # solo: stage+write this 2-mt unit on its own (tail units,
                # so the last y DMA is 2 mt instead of a whole strip)
                strip = u // 2
                key = (pr, strip)
                if solo:
                    ysb = ysb_p.tile([128, 4, C], BF16, tag="ysb", name="ysb")
                elif key not in ysb_open:
                    ysb = ysb_p.tile([128, 4, C], BF16, tag="ysb", name="ysb")
                    ysb_open[key] = ysb
                else:
                    ysb = ysb_open[key]
                for j, mt in enumerate((2 * u, 2 * u + 1)):
                    for ns in range(2):
                        yp = ps_a.tile([128, 384], F32, tag="ps_a")
                        if solo and pr == 2 and u in (6, 7) \
                                and (u - 6) * 256 in stg_out:
                            # split-K: even head from attn_sb partitions
                            # 0-63, odd head from the unshifted stg tile
                            # against the replicated odd wp rows
                            c0u = (u - 6) * 256
                            mtsl = slice(mt * 128, (mt + 1) * 128)
                            loc = (mt - (12 if u == 6 else 14)) * 128
                            nc.tensor.matmul(
                                yp,
                                lhsT=attn_sb[0:D, 2, mtsl],
                                rhs=wp_sb[0:D, 2, ns * 384:(ns + 1) * 384],
                                start=True, stop=False,
                            )
                            nc.tensor.matmul(
                                yp,
                                lhsT=stg_out[c0u][:, loc:loc + 128],
                                rhs=wp_odd_sb[:, ns * 384:(ns + 1) * 384],
                                start=False, stop=True,
                            )
                        else:
                            nc.tensor.matmul(
                                yp,
                                lhsT=attn_sb[:, pr, mt * 128:(mt + 1) * 128],
                                rhs=wp_sb[:, pr, ns * 384:(ns + 1) * 384],
                            )
                        # GPSIMD can't read PSUM; DVE carries the copies,
                        # with ACT (idle at the tail) taking the solo units'
                        # second half so the drain isn't DVE-serialized
                        dst = ysb[:, (0 if solo else u % 2) * 2 + j,
                                  ns * 384:(ns + 1) * 384]
                        if solo and ns == 1:
                            nc.scalar.copy(dst, yp)
                        else:
                            nc.vector.tensor_copy(dst, yp)
                if solo:
                    r0 = u * 256
                    nc.sync.dma_start(
                        y[pr, r0:r0 + 256, :]
                        .rearrange("(m p) c -> p m c", p=128), ysb[:, 0:2, :])
                elif u % 2 == 1:
                    del ysb_open[key]
                    r0 = strip * 512
                    nc.sync.dma_start(
                        y[pr, r0:r0 + 512, :]
                        .rearrange("(m p) c -> p m c", p=128), ysb)

            # --- emission order = scheduler priority; the scheduler pulls
            # the highest-priority READY instruction whenever an engine
            # idles. Chase work (K/V tiles an attention window depends on,
            # the next window's q strip) and proj units are interleaved
            # into the attention stream at kt granularity so priority
            # matches each item's deadline: K-strip s of a pair lands ~4 kt
            # before its scores, q strips one window ahead, proj units
            # (2 mt tiles) fill the remaining exp-wait gaps.
            def qk(t, s):
                return ("qk", t, s)

            def pj(p, u):
                return ("pj", p, u)

            def emit_slack(items):
                for it in items:
                    if it[0] == "qk":
                        emit_qk_strip(it[1], it[2])
                    elif it[0] == "v":
                        emit_v(it[1])
                    elif it[0] == "pj6":
                        emit_proj_unit(2, 6, solo=True)
                    else:
                        emit_proj_unit(it[1], it[2])

            K = HL // 2  # 3
            V = lambda mt: ("v", mt)
            # slots1: K-strip/q-strip chase inside the scores+exp stream,
            # placed >= 4 kv tiles (or a full window) ahead of the scores
            # that need them. slots2: v-tile chase and proj units inside
            # the (lagging) attnV stream. Keeping every list inside its
            # window prevents leftover slack from outranking the next
            # window's scores at a boundary.
            S1 = {
                (0, 0): {1: [qk(K, 1)], 5: [qk(K, 2)], 9: [qk(K, 3)],
                         13: [qk(0, 1)]},
                (0, 1): {4: [qk(0, 2)], 10: [qk(K + 1, 0)]},
                (0, 2): {4: [qk(0, 3)], 10: [qk(K + 1, 1)]},
                (0, 3): {4: [qk(K + 1, 2)], 10: [qk(K + 1, 3)]},
                (1, 0): {10: [qk(K + 2, 0)]},
                (1, 1): {10: [qk(K + 2, 1)]},
                (1, 2): {4: [qk(K + 2, 2)]},
                (1, 3): {4: [qk(K + 2, 3)], 10: [qk(2, 0)]},
                (2, 0): {4: [qk(2, 1)]},
                (2, 1): {4: [qk(2, 2)]},
                (2, 2): {4: [qk(2, 3)]},
            }
            S2 = {
                (0, 0): {0: [V(4)], 1: [V(5)], 2: [V(6)], 3: [V(7)],
                         4: [V(8)], 5: [V(9)], 6: [V(10)], 7: [V(11)],
                         8: [V(12)], 9: [V(13)], 10: [V(14)], 11: [V(15)]},
                (0, 2): {0: [qk(1, 0)]},
                (0, 3): {0: [qk(1, 1)], 5: [pj(0, 0)]},
                (1, 0): {0: [qk(1, 2)], 5: [pj(0, 1)]},
                (1, 1): {0: [qk(1, 3)], 5: [pj(0, 2)], 10: [pj(0, 3)]},
                (1, 2): {0: [pj(0, 4)], 5: [pj(0, 5)], 10: [pj(1, 0)]},
                (1, 3): {0: [pj(0, 6)], 5: [pj(0, 7)], 10: [pj(1, 1)]},
                (2, 0): {0: [pj(1, 2)], 5: [pj(1, 3)], 10: [pj(1, 4)]},
                (2, 1): {0: [pj(1, 5)], 5: [pj(1, 6)], 10: [pj(2, 0)],
                         13: [pj(2, 1)]},
                (2, 2): {0: [pj(1, 7)], 5: [pj(2, 2)], 10: [pj(2, 3)]},
            }

            emit_qk_strip(K, 0)   # k pair 0, kv strip 0
            emit_qk_strip(0, 0)   # q pair 0, strip 0
            for mt in range(4):
                emit_v(mt)
            wins = [(pr, qs) for pr in range(HL // 2) for qs in range(QS)]
            prev = None
            for pr, qs in wins:
                if pr == 2 and qs == 3:
                    ests_a = emit_scores_exp(pr, qs, 0, 256)
                    emit_attnv_norm(*prev[:2], prev[2],
                                    slots=S2.get(prev[:2]))
                    ests_b = emit_scores_exp(pr, qs, 256, 256)
                    emit_attnv_norm(pr, qs, ests_a, 0, 256)
                    emit_slack([pj(2, 4), pj(2, 5)])
                    emit_attnv_norm(pr, qs, ests_b, 256, 256)
                    emit_proj_unit(2, 6, solo=True)
                    emit_proj_unit(2, 7, solo=True)
                    prev = None
                    continue
                ests = emit_scores_exp(pr, qs, slots1=S1.get((pr, qs)))
                if prev is not None:
                    emit_attnv_norm(*prev[:2], prev[2],
                                    slots=S2.get(prev[:2]))
                prev = (pr, qs, ests)

    nc.compile()
    return nc


def _get_nc():
    if "nc" not in _CACHE:
        _CACHE["nc"] = _build()
    return _CACHE["nc"]


def _prep_inputs(x, w_qkv, w_proj):
    """Per-core input dicts. Core c: batch c//2, head-half c%2."""
    wq, wk, wv = w_qkv[0:C], w_qkv[C:2 * C], w_qkv[2 * C:3 * C]
    in_maps = []
    for core in range(NCORES):
        b, p = divmod(core, 2)
        heads = [p * HL + j for j in range(HL)]
        qk_rows = np.concatenate(
            [wq[h * D:(h + 1) * D] for h in heads]
            + [wk[h * D:(h + 1) * D] for h in heads], axis=0)   # [768, C]
        v_rows = np.concatenate(
            [wv[h * D:(h + 1) * D] for h in heads], axis=0)     # [384, C]
        p_cols = np.concatenate(
            [w_proj[:, h * D:(h + 1) * D] for h in heads], axis=1)  # [C, 384]
        in_maps.append({
            "identT": np.eye(128, dtype=ml_dtypes.bfloat16),
            "xT": np.ascontiguousarray(x[b].T).astype(ml_dtypes.bfloat16),
            "wqkT": np.ascontiguousarray(qk_rows.T).astype(ml_dtypes.bfloat16),
            "wvT": np.ascontiguousarray(v_rows.T).astype(ml_dtypes.bfloat16),
            "wpT": np.ascontiguousarray(p_cols.T).astype(ml_dtypes.bfloat16),
        })
    return in_maps


def kernel(x, w_qkv, w_proj, b_proj, _trace=False):
    x = np.asarray(x, dtype=np.float32)
    w_qkv = np.asarray(w_qkv, dtype=np.float32)
    w_proj = np.asarray(w_proj, dtype=np.float32)
    b_proj = np.asarray(b_proj, dtype=np.float32)

    nc = _get_nc()
    in_maps = _prep_inputs(x, w_qkv, w_proj)
    # retry: transient NRT_EXEC_UNIT_UNRECOVERABLE has been observed once
    # on a first attempt and succeeded immediately on retry
    last_exc = None
    for _attempt in range(3):
        try:
            res = run_bass_kernel_spmd(nc, in_maps,
                                       core_ids=list(range(NCORES)),
                                       trace=_trace)
            break
        except Exception as e:
            last_exc = e
    else:
        raise last_exc
    _CACHE["last_results"] = res

    out = np.empty((B, N, C), dtype=np.float32)
    for b in range(B):
        out[b] = (res.results[2 * b]["y"].astype(np.float32).sum(0)
                  + res.results[2 * b + 1]["y"].astype(np.float32).sum(0)
                  + b_proj)
    return out


# revision 86
# speedup vs baseline: 1.0287x; 1.0287x over previous
"""Multi-head attention (B=4,N=2048,C=768,H=12) on 8 trn2 NeuronCores.

Sharding: data-parallel over B (4 batches x 2 cores each), tensor-parallel
over heads (6 heads per core). Each core:
  - QKV projection for its 6 heads (bf16 inputs/weights, fp32 accumulate;
    bf16 x adds ~2e-3 rel err and halves the DMA fill on the critical path)
  - transposed scores st[kv, q] (f32r, contraction D=64), two heads
    row-packed into PE partitions 0-63 / 64-127
  - exp on ScalarE (scale fused), bf16 output
  - attn@V in [q, d] layout: est is the stationary operand and V (with a
    ones-column for the softmax denominator) moves, ap=65 — half the PE
    cost of the [d, q] form; the denominator lands on the free axis so
    normalize is a per-partition reciprocal + tensor_scalar_mul (no
    partition broadcast), and a small PE transpose (identity input) puts
    each head at its proj partitions (odd head at base 64, no shift DMA).
    Each accumulator runs as its OWN kt-loop in its own ring buffer:
    start=True zeroes beyond the written region, so interleaved
    accumulation groups in one PSUM bank corrupt each other (measured).
  - output projection (bf16) -> per-pair partial y (bf16) to DRAM
Host sums the six partials per batch (3 pairs x 2 cores) and adds bias.

Schedule (the TileScheduler pulls the highest-priority READY instruction
whenever an engine idles, so emission order is a priority schedule):
  - The exp stream on ScalarE is the critical path; every strip's
    scores+exp are emitted first (phase 1), and its attn@V+normalize
    (phase 2) is emitted one window later (software pipelining over a
    26-deep est ring), so a window's drain never outranks the next
    window's scores.
  - The DMA fill is strip-ordered with few large transfers (the issue
    front-end costs ~0.65us each); pair 0's attention chases the fill.
  - Chase work (K strips 4 kv-tiles ahead of their scores, q strips one
    window ahead) and proj units ride in indexed slots of the two
    streams; proj of pair N-1 fills pair N's exp-wait gaps.
  - The final strip is split 2x256 and the last y writes are 2-mt solo
    DMAs whose staging copies go to DVE and ACT in parallel.
"""

import sys

import numpy as np
import ml_dtypes

_REPO = "/opt/trn_rl_repo"
if _REPO not in sys.path:
    sys.path.insert(0, _REPO)

import concourse.bacc as bacc
import concourse.mybir as mybir
import concourse.tile as tile
from concourse.bass_utils import run_bass_kernel_spmd

B, N, C, H, D = 4, 2048, 768, 12, 64
HL = H // 2          # heads per core
SCALE = D ** -0.5
NCORES = 8
KT_C = C // 128      # 6 contraction tiles over C
QS = N // 512        # 4 query strips
KVT = N // 128       # 16 kv tiles

F32 = mybir.dt.float32
F32R = mybir.dt.float32r
BF16 = mybir.dt.bfloat16
EXP = mybir.ActivationFunctionType.Exp

_CACHE = {}


def _build():
    nc = bacc.Bacc("TRN2", target_bir_lowering=False, debug=False,
                   num_devices=NCORES)
    xT = nc.dram_tensor("xT", [C, N], BF16, kind="ExternalInput").ap()
    wqkT = nc.dram_tensor("wqkT", [C, 2 * HL * D], BF16, kind="ExternalInput").ap()
    wvT = nc.dram_tensor("wvT", [C, HL * D], BF16, kind="ExternalInput").ap()
    wpT = nc.dram_tensor("wpT", [HL * D, C], BF16, kind="ExternalInput").ap()
    identT = nc.dram_tensor("identT", [128, 128], BF16, kind="ExternalInput").ap()
    y = nc.dram_tensor("y", [HL // 2, N, C], BF16, kind="ExternalOutput").ap()

    with tile.TileContext(nc) as tc:
        with (
            tc.tile_pool(name="singles", bufs=1) as singles,
            tc.tile_pool(name="ps_a", bufs=2, space="PSUM") as ps_a,
            tc.tile_pool(name="ps_st", bufs=2, space="PSUM") as ps_st,
            tc.tile_pool(name="ps_out", bufs=2, space="PSUM") as ps_out,
            tc.tile_pool(name="est", bufs=32) as est_p,
            tc.tile_pool(name="rec", bufs=4) as rec_p,
            tc.tile_pool(name="rb", bufs=3) as rb_p,
            tc.tile_pool(name="ysb", bufs=3) as ysb_p,
        ):
            xT_sb = singles.tile([128, KT_C, N], BF16)
            wqk_sb = singles.tile([128, KT_C, 2 * HL * D], BF16)
            wv_sb = singles.tile([128, KT_C, HL * D], BF16)
            wp_sb = singles.tile([128, HL // 2, C], BF16)
            qk_sb = singles.tile([128, 2 * (HL // 2), N], F32R)
            # per head: [V | ones]; the ones column yields the softmax denom
            v_sb = singles.tile([128, KVT, HL // 2, 2, D + 1], BF16)
            # attention output in proj-ready pair layout: [128, pair, N]
            attn_sb = singles.tile([128, HL // 2, N], BF16)
            # pair-2 odd head's proj rows replicated at partitions 0-63 so
            # the final sub-strip can project without the partition-shift
            # DMA (split-K accumulation instead)
            wp_odd_sb = singles.tile([64, C], BF16)
            ident_sb = singles.tile([128, 128], BF16)
            # scratch for the PE warmup matmuls; memset before the DMAs so
            # the warmups only wait on this one short DVE op
            nc.vector.memset(attn_sb[:, 0, 0:640], 0.0)

            # --- DMA fill, strip-ordered so attention pair 0 can chase it.
            # Few, large transfers: the DMA descriptor front-end costs
            # ~0.6us per dma_start regardless of size. wqk slices for the
            # k-tile (t=3) and q-tile (t=0) of pair 0 come first.
            t0c, t3c = 0, (HL // 2) * 128
            wqkT_k = wqkT.rearrange("(kt p) c -> p kt c", p=128)
            wvT_k = wvT.rearrange("(kt p) c -> p kt c", p=128)
            nc.sync.dma_start(wqk_sb[:, :, t3c:t3c + 128],
                              wqkT_k[:, :, t3c:t3c + 128])
            # each xT strip is one transfer; the DMA issue front-end costs
            # ~0.65us per dma_start, so fewer issues beat finer chase
            # granularity for time-to-first-exp
            xT_k = xT.rearrange("(kt p) n -> p kt n", p=128)
            nc.sync.dma_start(xT_sb[:, :, 0:512], xT_k[:, :, 0:512])
            nc.sync.dma_start(wqk_sb[:, :, t0c:t0c + 128],
                              wqkT_k[:, :, t0c:t0c + 128])
            nc.sync.dma_start(wv_sb, wvT_k)
            for s in range(1, QS):
                sl = slice(s * 512, (s + 1) * 512)
                nc.sync.dma_start(xT_sb[:, :, sl], xT_k[:, :, sl])
            nc.sync.dma_start(wqk_sb[:, :, 128:384], wqkT_k[:, :, 128:384])
            nc.sync.dma_start(wqk_sb[:, :, 512:768], wqkT_k[:, :, 512:768])
            nc.sync.dma_start(wp_sb, wpT.rearrange("(pr p) c -> p pr c", p=128))
            nc.sync.dma_start(wp_odd_sb, wpT[2 * 128 + 64:3 * 128, :])
            nc.sync.dma_start(ident_sb, identT)
            nc.vector.memset(v_sb[:, :, :, :, D:D + 1], 1.0)

            # warm the ACT exp table so the ~1.3us ACT_TABLE_LOAD is off the
            # first real exp's critical path
            warm_in = rec_p.tile([1, 2], F32, tag="warm")
            warm_out = rec_p.tile([1, 2], BF16, tag="warmo")
            nc.vector.memset(warm_in, 0.0)
            nc.scalar.activation(warm_out, warm_in, EXP, scale=SCALE)

            # dependency-free matmuls ramp the PE clock to 2.4GHz during the
            # initial DMA window so the first real matmuls run at full speed
            for _ in range(7):
                warm_ps = ps_out.tile([128, 512], F32, tag="out")
                nc.tensor.matmul(warm_ps, lhsT=attn_sb[:, 0, 0:128],
                                 rhs=attn_sb[:, 0, 128:640])

            def emit_qk_strip(t, qs):
                qsl = slice(qs * 512, (qs + 1) * 512)
                ps = ps_a.tile([128, 512], F32, tag="ps_a")
                for kt in range(KT_C):
                    nc.tensor.matmul(
                        ps,
                        lhsT=wqk_sb[:, kt, t * 128:(t + 1) * 128],
                        rhs=xT_sb[:, kt, qsl],
                        start=(kt == 0), stop=(kt == KT_C - 1),
                    )
                nc.vector.tensor_copy(qk_sb[:, t, qsl], ps)

            def emit_v(mt):
                ps = ps_a.tile([128, HL * D], F32, tag="ps_a")
                for kt in range(KT_C):
                    nc.tensor.matmul(
                        ps,
                        lhsT=xT_sb[:, kt, mt * 128:(mt + 1) * 128],
                        rhs=wv_sb[:, kt, :],
                        start=(kt == 0), stop=(kt == KT_C - 1),
                    )
                nc.vector.tensor_copy(
                    v_sb[:, mt, :, :, 0:D],
                    ps.rearrange("p (pr two d) -> p pr two d", pr=HL // 2, two=2),
                )

            stg_out = {}

            def emit_scores_exp(pr, qs, c0=0, cw=512, slots1=None):
                # scores + exp for every kv tile of query sub-range
                # [c0, c0+cw) of strip qs. The exp stream is the kernel's
                # critical path; slots1 interleaves the K/q-strip chase a
                # few kv tiles ahead of the scores that consume it.
                tq, tk = pr, HL // 2 + pr
                qsl = slice(qs * 512 + c0, qs * 512 + c0 + cw)
                ests = []
                for kt in range(KVT):
                    # both heads' scores into one 2-bank tile, one exp
                    st = ps_st.tile([128, 2, 512], F32, tag="st")
                    for half in range(2):
                        p0, p1 = half * 64, (half + 1) * 64
                        nc.tensor.matmul(
                            st[:, half, 0:cw],
                            lhsT=qk_sb[p0:p1, tk, kt * 128:(kt + 1) * 128],
                            rhs=qk_sb[p0:p1, tq, qsl],
                        )
                    est = est_p.tile([128, 2, 512], BF16, tag="est")
                    nc.scalar.activation(est[:, :, 0:cw], st[:, :, 0:cw],
                                         EXP, scale=SCALE)
                    ests.append(est)
                    if slots1 and kt in slots1:
                        emit_slack(slots1[kt])
                return ests

            def emit_attnv_norm(pr, qs, ests, c0=0, cw=512,
                                skip_shift=False, slots=None):
                # attn@V in [q, d] layout: est is the stationary operand,
                # V moves (ap=65), so attn@V costs half of the [d, q]
                # form in PE time; the softmax denominator lands on the
                # FREE axis so normalize is a native per-partition
                # tensor_scalar_mul (no partition broadcast), and a small
                # PE transpose drops each head at its proj partitions
                # (odd head legally at base 64) with no shift DMA.
                # Each accumulator runs as its OWN kt-loop in its own
                # ring buffer: a matmul with start=True zeroes beyond the
                # written region, so interleaved accumulation groups in
                # one PSUM bank corrupt each other (measured on hardware).
                for j in range(cw // 128):
                    qcol = qs * 512 + c0 + j * 128
                    for half in range(2):
                        out_q = ps_out.tile([128, 512], F32, tag="out")
                        for kt in range(KVT):
                            nc.tensor.matmul(
                                out_q[:, 0:65],
                                lhsT=ests[kt][:, half,
                                              j * 128:(j + 1) * 128],
                                rhs=v_sb[:, kt, pr, half, :],
                                start=(kt == 0), stop=(kt == KVT - 1),
                            )
                        rec_q = rec_p.tile([128, 1], F32, tag="recq")
                        nc.vector.reciprocal(rec_q, out_q[:, 64:65])
                        aq = rb_p.tile([128, 64], BF16, tag="aq")
                        nc.vector.tensor_scalar_mul(aq, out_q[:, 0:64],
                                                    rec_q)
                        tp = ps_a.tile([128, 128], BF16, tag="ps_a",
                                       name="tp")
                        nc.tensor.transpose(
                            tp[half * 64:(half + 1) * 64, :], aq, ident_sb)
                        nc.vector.tensor_copy(
                            attn_sb[half * 64:(half + 1) * 64, pr,
                                    qcol:qcol + 128],
                            tp[half * 64:(half + 1) * 64, :])
                    if slots:
                        for kk in range(4 * j, 4 * j + 4):
                            if kk in slots:
                                emit_slack(slots[kk])

            # proj: one DMA per 4-mt strip (the y-write descriptor front-end
            # costs ~0.6us per dma_start, so per-mt writes would throttle
            # the tail); a proj "unit" is 2 mt tiles, two units share a ysb
            ysb_open = {}

            def emit_proj_unit(pr, u, solo=False):
                # solo: stage+write this 2-mt unit on its own (tail units,
                # so the last y DMA is 2 mt instead of a whole strip)
                strip = u // 2
                key = (pr, strip)
                if solo:
                    ysb = ysb_p.tile([128, 4, C], BF16, tag="ysb", name="ysb")
                elif key not in ysb_open:
                    ysb = ysb_p.tile([128, 4, C], BF16, tag="ysb", name="ysb")
                    ysb_open[key] = ysb
                else:
                    ysb = ysb_open[key]
                for j, mt in enumerate((2 * u, 2 * u + 1)):
                    for ns in range(2):
                        yp = ps_a.tile([128, 384], F32, tag="ps_a")
                        if solo and pr == 2 and u in (6, 7) \
                                and (u - 6) * 256 in stg_out:
                            # split-K: even head from attn_sb partitions
                            # 0-63, odd head from the unshifted stg tile
                            # against the replicated odd wp rows
                            c0u = (u - 6) * 256
                            mtsl = slice(mt * 128, (mt + 1) * 128)
                            loc = (mt - (12 if u == 6 else 14)) * 128
                            nc.tensor.matmul(
                                yp,
                                lhsT=attn_sb[0:D, 2, mtsl],
                                rhs=wp_sb[0:D, 2, ns * 384:(ns + 1) * 384],
                                start=True, stop=False,
                            )
                            nc.tensor.matmul(
                                yp,
                                lhsT=stg_out[c0u][:, loc:loc + 128],
                                rhs=wp_odd_sb[:, ns * 384:(ns + 1) * 384],
                                start=False, stop=True,
                            )
                        else:
                            nc.tensor.matmul(
                                yp,
                                lhsT=attn_sb[:, pr, mt * 128:(mt + 1) * 128],
                                rhs=wp_sb[:, pr, ns * 384:(ns + 1) * 384],
                            )
                        # GPSIMD can't read PSUM; DVE carries the copies,
                        # with ACT (idle at the tail) taking the solo units'
                        # second half so the drain isn't DVE-serialized
                        dst = ysb[:, (0 if solo else u % 2) * 2 + j,
                                  ns * 384:(ns + 1) * 384]
                        if solo and ns == 1:
                            nc.scalar.copy(dst, yp)
                        else:
                            nc.vector.tensor_copy(dst, yp)
                if solo:
                    r0 = u * 256
                    nc.sync.dma_start(
                        y[pr, r0:r0 + 256, :]
                        .rearrange("(m p) c -> p m c", p=128), ysb[:, 0:2, :])
                elif u % 2 == 1:
                    del ysb_open[key]
                    r0 = strip * 512
                    nc.sync.dma_start(
                        y[pr, r0:r0 + 512, :]
                        .rearrange("(m p) c -> p m c", p=128), ysb)

            # --- emission order = scheduler priority; the scheduler pulls
            # the highest-priority READY instruction whenever an engine
            # idles. Chase work (K/V tiles an attention window depends on,
            # the next window's q strip) and proj units are interleaved
            # into the attention stream at kt granularity so priority
            # matches each item's deadline: K-strip s of a pair lands ~4 kt
            # before its scores, q strips one window ahead, proj units
            # (2 mt tiles) fill the remaining exp-wait gaps.
            def qk(t, s):
                return ("qk", t, s)

            def pj(p, u):
                return ("pj", p, u)

            def emit_slack(items):
                for it in items:
                    if it[0] == "qk":
                        emit_qk_strip(it[1], it[2])
                    elif it[0] == "v":
                        emit_v(it[1])
                    elif it[0] == "pj6":
                        emit_proj_unit(2, 6, solo=True)
                    else:
                        emit_proj_unit(it[1], it[2])

            K = HL // 2  # 3
            V = lambda mt: ("v", mt)
            # slots1: K-strip/q-strip chase inside the scores+exp stream,
            # placed >= 4 kv tiles (or a full window) ahead of the scores
            # that need them. slots2: v-tile chase and proj units inside
            # the (lagging) attnV stream. Keeping every list inside its
            # window prevents leftover slack from outranking the next
            # window's scores at a boundary.
            S1 = {
                (0, 0): {1: [qk(K, 1)], 5: [qk(K, 2)], 9: [qk(K, 3)],
                         13: [qk(0, 1)]},
                (0, 1): {4: [qk(0, 2)], 10: [qk(K + 1, 0)]},
                (0, 2): {4: [qk(0, 3)], 10: [qk(K + 1, 1)]},
                (0, 3): {4: [qk(K + 1, 2)], 10: [qk(K + 1, 3)]},
                (1, 0): {10: [qk(K + 2, 0)]},
                (1, 1): {10: [qk(K + 2, 1)]},
                (1, 2): {4: [qk(K + 2, 2)]},
                (1, 3): {4: [qk(K + 2, 3)], 10: [qk(2, 0)]},
                (2, 0): {4: [qk(2, 1)]},
                (2, 1): {4: [qk(2, 2)]},
                (2, 2): {4: [qk(2, 3)]},
            }
            S2 = {
                (0, 0): {0: [V(4)], 1: [V(5)], 2: [V(6)], 3: [V(7)],
                         4: [V(8)], 5: [V(9)], 6: [V(10)], 7: [V(11)],
                         8: [V(12)], 9: [V(13)], 10: [V(14)], 11: [V(15)]},
                (0, 2): {0: [qk(1, 0)]},
                (0, 3): {0: [qk(1, 1)], 5: [pj(0, 0)]},
                (1, 0): {0: [qk(1, 2)], 5: [pj(0, 1)]},
                (1, 1): {0: [qk(1, 3)], 5: [pj(0, 2)], 10: [pj(0, 3)]},
                (1, 2): {0: [pj(0, 4)], 5: [pj(0, 5)], 10: [pj(1, 0)]},
                (1, 3): {0: [pj(0, 6)], 5: [pj(0, 7)], 10: [pj(1, 1)]},
                (2, 0): {0: [pj(1, 2)], 5: [pj(1, 3)], 10: [pj(1, 4)]},
                (2, 1): {0: [pj(1, 5)], 5: [pj(1, 6)], 10: [pj(2, 0)],
                         13: [pj(2, 1)]},
                (2, 2): {0: [pj(1, 7)], 5: [pj(2, 2)], 10: [pj(2, 3)]},
            }

            emit_qk_strip(K, 0)   # k pair 0, kv strip 0
            emit_qk_strip(0, 0)   # q pair 0, strip 0
            for mt in range(4):
                emit_v(mt)
            wins = [(pr, qs) for pr in range(HL // 2) for qs in range(QS)]
            prev = None
            for pr, qs in wins:
                if pr == 2 and qs == 3:
                    ests_a = emit_scores_exp(pr, qs, 0, 256)
                    emit_attnv_norm(*prev[:2], prev[2],
                                    slots=S2.get(prev[:2]))
                    ests_b = emit_scores_exp(pr, qs, 256, 256)
                    emit_attnv_norm(pr, qs, ests_a, 0, 256)
                    emit_slack([pj(2, 4), pj(2, 5)])
                    emit_attnv_norm(pr, qs, ests_b, 256, 256)
                    emit_proj_unit(2, 6, solo=True)
                    emit_proj_unit(2, 7, solo=True)
                    prev = None
                    continue
                ests = emit_scores_exp(pr, qs, slots1=S1.get((pr, qs)))
                if prev is not None:
                    emit_attnv_norm(*prev[:2], prev[2],
                                    slots=S2.get(prev[:2]))
                prev = (pr, qs, ests)

    nc.compile()
    return nc


def _get_nc():
    if "nc" not in _CACHE:
        _CACHE["nc"] = _build()
    return _CACHE["nc"]


def _prep_inputs(x, w_qkv, w_proj):
    """Per-core input dicts. Core c: batch c//2, head-half c%2."""
    wq, wk, wv = w_qkv[0:C], w_qkv[C:2 * C], w_qkv[2 * C:3 * C]
    in_maps = []
    for core in range(NCORES):
        b, p = divmod(core, 2)
        heads = [p * HL + j for j in range(HL)]
        qk_rows = np.concatenate(
            [wq[h * D:(h + 1) * D] for h in heads]
            + [wk[h * D:(h + 1) * D] for h in heads], axis=0)   # [768, C]
        v_rows = np.concatenate(
            [wv[h * D:(h + 1) * D] for h in heads], axis=0)     # [384, C]
        p_cols = np.concatenate(
            [w_proj[:, h * D:(h + 1) * D] for h in heads], axis=1)  # [C, 384]
        in_maps.append({
            "identT": np.eye(128, dtype=ml_dtypes.bfloat16),
            "xT": np.ascontiguousarray(x[b].T).astype(ml_dtypes.bfloat16),
            "wqkT": np.ascontiguousarray(qk_rows.T).astype(ml_dtypes.bfloat16),
            "wvT": np.ascontiguousarray(v_rows.T).astype(ml_dtypes.bfloat16),
            "wpT": np.ascontiguousarray(p_cols.T).astype(ml_dtypes.bfloat16),
        })
    return in_maps


def kernel(x, w_qkv, w_proj, b_proj, _trace=False):
    x = np.asarray(x, dtype=np.float32)
    w_qkv = np.asarray(w_qkv, dtype=np.float32)
    w_proj = np.asarray(w_proj, dtype=np.float32)
    b_proj = np.asarray(b_proj, dtype=np.float32)

    nc = _get_nc()
    in_maps = _prep_inputs(x, w_qkv, w_proj)
    # retry: transient NRT_EXEC_UNIT_UNRECOVERABLE has been observed once
    # on a first attempt and succeeded immediately on retry
    last_exc = None
    for _attempt in range(3):
        try:
            res = run_bass_kernel_spmd(nc, in_maps,
                                       core_ids=list(range(NCORES)),
                                       trace=_trace)
            break
        except Exception as e:
            last_exc = e
    else:
        raise last_exc
    _CACHE["last_results"] = res

    out = np.empty((B, N, C), dtype=np.float32)
    for b in range(B):
        out[b] = (res.results[2 * b]["y"].astype(np.float32).sum(0)
                  + res.results[2 * b + 1]["y"].astype(np.float32).sum(0)
                  + b_proj)
    return out
